# revision 23
# baseline (speedup 1.0000x reference)
"""Llama4TextExperts MoE grouped-GEMM kernel for 8 Trainium2 NeuronCores.

Expert-parallel: core e owns expert e and the pre-sorted token block
hidden_states[e*4096:(e+1)*4096]. No collectives needed.

The host pre-packs all operands into bf16 device layouts so the PE does
nothing but the 6144 GEMM matmuls (no on-chip transposes):
  xt [4, 128, 16, 1024]  xt[c,p,k,t] = x[c*1024+t, k*128+p]  (xT, chunked)
  w1 [32, 128, 2048]     w1[mp*2+gu] = W1 column block, k-tiled, contiguous
  w2 [128, 16, 2048]     w2[p,k2,h]  = W2[k2*128+p, h]

Per-core pipeline over 4 token chunks of TC=1024:
  mm1: for each of 16 gate/up column-block pairs, accumulate
       psg/psu [128,512] over k=16 (bf16 matmuls, f32 PSUM),
       ACT silu -> bf16, DVE mul -> actT bf16 [D-part, T free]
  mm2: actT block as stationary, W2 rows as moving -> natural [token, H]
       PSUM output; DVE copy -> SBUF, store DMA on the scalar queue.

Scheduling notes (all trace-verified):
- bf16 runs at the same 1 cycle/row as f32r but halves DMA/SBUF and makes
  LDWEIGHTS fully hide under the 512-cycle moving stream (f32r LS did not).
- One shared 8-bank PSUM pool: each accumulation group's bank is reused 8
  allocations later, giving evacuation a full mp/tb period of slack.
- Loads go on the sync-engine DMA ring, stores on the scalar-engine ring;
  W2 streams in quarters so it never starves the chunk-0 w1 prefetches on
  the HBM-saturated ring.
- 12 warm-up matmuls on a zero tile ride the initial DMA wait and pull the
  PE out of its low p-state right as the real stream begins.
End-to-end rel err ~4.4e-3 (gate is 2e-2).
"""

import numpy as np

try:
    import concourse.bass as bass  # noqa: F401
except ImportError:
    import sys

    sys.path.insert(0, "/opt/trn_rl_repo")

import ml_dtypes

import concourse.mybir as mybir
import concourse.tile as tile
from concourse import bacc
from concourse.bass_utils import run_bass_kernel_spmd

F32 = mybir.dt.float32
BF16 = mybir.dt.bfloat16
SILU = mybir.ActivationFunctionType.Silu
P = 128
NPBF = ml_dtypes.bfloat16

NCORES = 8
H_FULL = 2048  # hidden size
D_FULL = 2048  # expert intermediate size
T_TOTAL = 32768
T_CORE = T_TOTAL // NCORES  # 4096 tokens per expert/core


def emit_moe(nc, out_ap, xt_ap, w1_ap, w2_ap, T, H, D, TC):
    K1 = H // P  # contraction tiles for mm1
    MP = D // P  # gate/up column-block pairs
    K2 = D // P  # contraction tiles for mm2
    NCH = T // TC  # token chunks
    MMW = 512  # moving width = one PSUM bank of f32
    NHF = TC // MMW  # 512-wide column groups per chunk (2)
    NHQ = H // MMW  # mm2 output column groups (4)
    NTB = TC // P  # token blocks per chunk for mm2 (8)

    KG = 4  # k-tiles per xT sub-tile (split so mm1 starts after 1/4 of x)
    NKG = K1 // KG

    with tile.TileContext(nc) as tc:
        with (
            tc.tile_pool(name="w2sb", bufs=1) as w2p,
            tc.tile_pool(name="const", bufs=1) as constp,
            tc.tile_pool(name="xT", bufs=NKG) as xTp,
            tc.tile_pool(name="actT", bufs=1) as actTp,
            tc.tile_pool(name="w1", bufs=6) as w1p,
            tc.tile_pool(name="sil", bufs=2 * NHF) as silp,
            tc.tile_pool(name="ost", bufs=2) as ostp,
            tc.tile_pool(name="ps", bufs=8, space="PSUM") as psp,
        ):
            # ---- load helpers with explicit prefetch scheduling ----
            xts = {}

            def load_xt_kg(c, kg):
                t = xTp.tile([P, KG * TC], BF16, tag="xT", name=f"xT_{c}_{kg}")
                nc.sync.dma_start(
                    out=t[:].rearrange("p (k t) -> p k t", k=KG),
                    in_=xt_ap[c, :, kg * KG : (kg + 1) * KG, :],
                )
                xts[(c, kg)] = t

            def load_xt(c):
                if c >= NCH:
                    return
                for kg in range(NKG):
                    load_xt_kg(c, kg)

            w1s = {}

            def load_w1_one(c, mp, gu):
                t = w1p.tile([P, K1 * P], BF16, tag="w1", name=f"w1_{c}_{mp}_{gu}")
                nc.sync.dma_start(out=t[:], in_=w1_ap[mp * 2 + gu])
                w1s[(c, mp, gu)] = t

            def load_w1(c, mp):
                if c >= NCH or mp >= MP:
                    return
                load_w1_one(c, mp, 0)
                load_w1_one(c, mp, 1)

            # Startup order matters: the first matmul needs w1-gate(0,0) and
            # the first xT k-group, so they go first on the ring; W2 (needed
            # only at mm2) goes last.
            load_w1_one(0, 0, 0)
            load_xt_kg(0, 0)
            load_xt_kg(0, 1)
            load_w1_one(0, 0, 1)
            load_xt_kg(0, 2)
            load_xt_kg(0, 3)
            load_w1(0, 1)
            w2sb = w2p.tile([P, K2 * H], BF16, tag="w2", name="w2sb")

            def load_w2_quarter(q):
                # W2 is 8MB; issued whole it starves the chunk-0 w1
                # prefetches on the (HBM-saturated) ring, so stream it in
                # quarters threaded between mm1(0) weight loads.
                kq = K2 // 4
                nc.sync.dma_start(
                    out=w2sb[:, q * kq * H : (q + 1) * kq * H],
                    in_=w2_ap[:, q * kq : (q + 1) * kq, :].rearrange(
                        "p k h -> p (k h)"
                    ),
                )

            # Warm-up matmuls on a zero tile: they depend on nothing, so
            # they run during the initial x/w DMAs and pull the PE out of
            # its low p-state right as the real stream begins.
            zc = constp.tile([P, MMW], BF16, tag="zc", name="zc")
            nc.gpsimd.memset(zc[:], 0.0)
            psw = psp.tile([P, MMW], F32, tag="ps", name="psw")
            for i in range(12):
                nc.tensor.matmul(
                    psw[:], zc[:, :P], zc[:], start=True, stop=True
                )

            for c in range(NCH):
                t0 = c * TC
                xT = {kg: xts.pop((c, kg)) for kg in range(NKG)}

                # ---- mm1 + SwiGLU -> actT [D on partitions, T free] ----
                actT = actTp.tile([P, K2 * TC], BF16, tag="actT", name=f"actT_{c}")
                for mp in range(MP):
                    load_w1(c, mp + 2)
                    if c == 0 and mp in (6, 8, 10, 12):
                        load_w2_quarter((mp - 6) // 2)
                    w1g = w1s.pop((c, mp, 0))
                    w1u = w1s.pop((c, mp, 1))
                    psg = [psp.tile([P, MMW], F32, tag="ps", name=f"psg{i}") for i in range(NHF)]
                    psu = [psp.tile([P, MMW], F32, tag="ps", name=f"psu{i}") for i in range(NHF)]
                    for k in range(K1):
                        st = w1g[:, k * P : (k + 1) * P]
                        xk = xT[k // KG]
                        kk = k % KG
                        for hf in range(NHF):
                            nc.tensor.matmul(
                                psg[hf][:],
                                st,
                                xk[:, kk * TC + hf * MMW : kk * TC + (hf + 1) * MMW],
                                start=(k == 0),
                                stop=(k == K1 - 1),
                            )
                    sil = [silp.tile([P, MMW], BF16, tag="sil", name=f"sil{i}") for i in range(NHF)]
                    for hf in range(NHF):
                        nc.scalar.activation(sil[hf][:], psg[hf][:], SILU)
                    for k in range(K1):
                        st = w1u[:, k * P : (k + 1) * P]
                        xk = xT[k // KG]
                        kk = k % KG
                        for hf in range(NHF):
                            nc.tensor.matmul(
                                psu[hf][:],
                                st,
                                xk[:, kk * TC + hf * MMW : kk * TC + (hf + 1) * MMW],
                                start=(k == 0),
                                stop=(k == K1 - 1),
                            )
                    for hf in range(NHF):
                        nc.vector.tensor_mul(
                            actT[:, mp * TC + hf * MMW : mp * TC + (hf + 1) * MMW],
                            sil[hf][:],
                            psu[hf][:],
                        )

                # prefetch next chunk's x and first weight pairs while the
                # PE runs mm2 on this chunk
                load_xt(c + 1)
                load_w1(c + 1, 0)
                load_w1(c + 1, 1)

                # ---- mm2: actT stationary, W2 moving -> natural [t, H] ----
                for tb in range(NTB):
                    ps2 = [psp.tile([P, MMW], F32, tag="ps", name=f"ps2_{i}") for i in range(NHQ)]
                    for k2 in range(K2):
                        st = actT[:, k2 * TC + tb * P : k2 * TC + (tb + 1) * P]
                        for hq in range(NHQ):
                            nc.tensor.matmul(
                                ps2[hq][:],
                                st,
                                w2sb[:, k2 * H + hq * MMW : k2 * H + (hq + 1) * MMW],
                                start=(k2 == 0),
                                stop=(k2 == K2 - 1),
                            )
                    ost = ostp.tile([P, H], F32, tag="ost", name=f"ost{tb}")
                    if c == NCH - 1 and tb == NTB - 1:
                        # final block: copies split across ACT/DVE and the
                        # store issued per-hq so the kernel-tail drain is
                        # short transfers behind parallel copies
                        for hq in range(NHQ):
                            dst = ost[:, hq * MMW : (hq + 1) * MMW]
                            if hq % 2 == 0:
                                nc.vector.tensor_copy(dst, ps2[hq][:])
                            else:
                                nc.scalar.copy(dst, ps2[hq][:])
                            nc.scalar.dma_start(
                                out=out_ap[
                                    t0 + tb * P : t0 + (tb + 1) * P,
                                    hq * MMW : (hq + 1) * MMW,
                                ],
                                in_=ost[:, hq * MMW : (hq + 1) * MMW],
                            )
                    else:
                        # copies go on the DVE (idle during mm2) so the ACT
                        # queue holds only sils and the next chunk's mm1
                        # never waits on a sil stuck behind these
                        for hq in range(NHQ):
                            nc.vector.tensor_copy(
                                ost[:, hq * MMW : (hq + 1) * MMW], ps2[hq][:]
                            )
                        nc.scalar.dma_start(
                            out=out_ap[t0 + tb * P : t0 + (tb + 1) * P, :], in_=ost[:]
                        )


def build(T=T_CORE, H=H_FULL, D=D_FULL, TC=1024):
    nc = bacc.Bacc("TRN2", target_bir_lowering=False, debug=False)
    xt = nc.dram_tensor(
        "xt", [T // TC, P, H // P, TC], BF16, kind="ExternalInput"
    ).ap()
    w1 = nc.dram_tensor(
        "w1", [2 * D // P, P, H // P * P], BF16, kind="ExternalInput"
    ).ap()
    w2 = nc.dram_tensor("w2", [P, D // P, H], BF16, kind="ExternalInput").ap()
    out = nc.dram_tensor("out", [T, H], F32, kind="ExternalOutput").ap()
    emit_moe(nc, out, xt, w1, w2, T, H, D, TC)
    nc.compile()
    return nc


_NC_CACHE = {}


def _get_nc():
    if "nc" not in _NC_CACHE:
        _NC_CACHE["nc"] = build()
    return _NC_CACHE["nc"]


def _prep_core(args):
    """Host-side pack of one expert's operands into device layouts."""
    x_e, w1_e, w2_e = args
    T, H, D = T_CORE, H_FULL, D_FULL
    # xt[c, p, k, t] = x[c*TC + t, k*128+p], chunk-major so each chunk's
    # per-partition DMA segments are KG*TC contiguous
    TC = 1024
    xbf = x_e.astype(NPBF)
    xt = np.ascontiguousarray(
        xbf.reshape(T // TC, TC, H // P, P).transpose(0, 3, 2, 1)
    )
    # w1r[mp, gu, p, k, c] = W1[k*128+p, gu*D + mp*128 + c], flattened to
    # [32, 128, 2048]
    w1bf = w1_e.astype(NPBF)
    w1r = np.ascontiguousarray(
        w1bf.reshape(H // P, P, 2, D // P, P).transpose(3, 2, 1, 0, 4)
    ).reshape(2 * D // P, P, H // P * P)
    # w2r[p, k2, h] = W2[k2*128+p, h]
    w2bf = w2_e.astype(NPBF)
    w2r = np.ascontiguousarray(w2bf.reshape(D // P, P, H).transpose(1, 0, 2))
    return {"xt": xt, "w1": w1r, "w2": w2r}


def run_sharded(hidden_states, gate_up_proj, down_proj, trace=False, **kwargs):
    """Run on 8 cores; returns (full_output, BassKernelResults)."""
    hidden_states = np.asarray(hidden_states, dtype=np.float32)
    gate_up_proj = np.asarray(gate_up_proj, dtype=np.float32)
    down_proj = np.asarray(down_proj, dtype=np.float32)

    nc = _get_nc()
    in_maps = [
        _prep_core(
            (
                hidden_states[e * T_CORE : (e + 1) * T_CORE],
                gate_up_proj[e],
                down_proj[e],
            )
        )
        for e in range(NCORES)
    ]
    res = run_bass_kernel_spmd(
        nc, in_maps, core_ids=list(range(NCORES)), trace=trace, **kwargs
    )
    out = np.concatenate([res.results[e]["out"] for e in range(NCORES)], axis=0)
    return out, res


def kernel(hidden_states, gate_up_proj, down_proj):
    import os

    # The NTFF trace path needs antenv.axon_hooks, absent in this image;
    # make sure a stray BASS_TRACE env can't route us into it.
    os.environ["BASS_NEVER_TRACE"] = "1"
    try:
        out, _ = run_sharded(hidden_states, gate_up_proj, down_proj)
    finally:
        del os.environ["BASS_NEVER_TRACE"]
    return out
